# revision 15
# baseline (speedup 1.0000x reference)
"""Haar DWT decoder (2-level inverse, zero details) as a Trainium2 Bass kernel.

out[b, c, j, k] = z[b].reshape(C, 128, 128)[c, j//4, k//4] * 0.25
i.e. a 4x4 nearest-neighbor upsample scaled by 1/4.

Data-parallel over batch: 128 samples -> 16 per core on 8 NeuronCores.

Per-core shape of the problem: read 3 MiB of z, write 48 MiB of output
through 16 SDMA engines at ~26.5 GB/s each (~424 GB/s aggregate), so the
floor is ~122 us of streaming plus the pipeline lead-in.

All DMA (loads and stores) goes through the two HWDGE rings (sync +
scalar). SWDGE (gpsimd) is deliberately unused: its descriptor rings
live on SBUF partitions whose AXI ports are shared with SDMA engines
7/15, and descriptor fetches from those rings are the known cause of
the "engines 7/15 run ~18% slow" straggler mode that adds ~20 us of
tail. HWDGE has no SBUF descriptor ring (and ~0.6 us vs ~1 us
first-byte latency). Loads ride the store rings' FIFO: six are issued
up front, then one after each store — a load is only ~0.5 us of ring
time, and compute stays ~20 us ahead of the ring cadence, so they never
stall the stores.
"""

import numpy as np

import concourse.bass as bass
import concourse.mybir as mybir
import concourse.tile as tile
from concourse.bass_utils import run_bass_kernel_spmd

# The walrus build in this container rejects instructions carrying more than
# one sync-wait command (codegen: "Too many sync wait commands" — observed on
# a Drain with 3 waits and a DMACopy with 2). Tile freely attaches several
# waits to one instruction, so after tracing we split the excess onto NOPs
# inserted just before the instruction on the same engine; sequential
# dispatch on one engine makes that equivalent.
_MAX_WAITS = 1


def _split_excess_waits(nc: bass.Bass) -> None:
    for f in nc.m.functions:
        for bb in f.blocks:
            insns = bb.instructions
            # Iterate over a snapshot; mutate the live list via insert.
            for ins in list(insns):
                si = ins.sync_info
                if si is None or not si.on_wait or len(si.on_wait) <= _MAX_WAITS:
                    continue
                waits = list(si.on_wait)
                keep = waits[-_MAX_WAITS:]
                spill = waits[:-_MAX_WAITS]
                pos = insns.index(ins)
                nops = []
                for i in range(0, len(spill), _MAX_WAITS):
                    nop = nc.engines[ins.engine].nop(nofuse=True).ins
                    # nop() appended itself to the current bb; pull it out.
                    cur = nc.cur_bb.bb.instructions
                    assert cur[-1] is nop
                    cur.pop()
                    nop.sync_info = mybir.SyncInfo(
                        on_wait=spill[i : i + _MAX_WAITS], on_update=[]
                    )
                    nops.append(nop)
                insns[pos:pos] = nops
                ins.sync_info = mybir.SyncInfo(
                    on_wait=keep, on_update=list(si.on_update)
                )

# Problem constants (hardcoded: module config out_shape=(3,512,512), levels=2)
BATCH = 128
C = 3
CAH = 128  # coarse-approximation spatial dims
CAW = 128
S = 4      # 2**levels upsample factor
H = 512
W = 512
N_CORES = 8
B_SHARD = BATCH // N_CORES  # 16

# Loads issued ahead of the compute loop (ring FIFO keeps them ~6 samples
# ahead of the stores, which is far more than compute needs).
PRELOAD = 3  # pairs

F32 = mybir.dt.float32


def _build_nc(b_shard: int = B_SHARD) -> bass.Bass:
    nc = bass.Bass("TRN2", target_bir_lowering=False, debug=False)
    z = nc.dram_tensor("z", [b_shard, C * CAH * CAW], F32, kind="ExternalInput").ap()
    # Output is declared FLAT per sample and reshaped to (C, H, W) in numpy:
    # a coarse row r = c*128+jc owns exactly the 2048 contiguous output
    # floats at offset 2048*r, so a partition that owns a contiguous flat
    # input chunk also owns a contiguous flat output chunk.
    out = nc.dram_tensor("out", [b_shard, C * H * W], F32, kind="ExternalOutput").ap()

    # Samples are processed in PAIRS treated as one flat 6 MiB block:
    # partition p owns pair-input floats [768p, 768(p+1)) — 3 KiB contiguous
    # load descriptors (2x the single-sample layout, which was stuck at
    # 1536 B / 22.5 GB/s) — and the corresponding 48 KiB of pair output
    # (coarse rows 6p..6p+5; partitions 0..63 are the even sample, 64..127
    # the odd one). Each pair is stored as TWO 3 MiB half-stores (rows
    # 6p+3h..6p+3h+2), keeping the proven 24 KiB store descriptors and the
    # 3 MiB pipeline granularity.
    n_pairs = b_shard // 2
    zp = z.rearrange("(u two) x -> u (two x)", two=2)
    op = out.rearrange("(u two) x -> u (two x)", two=2)
    QH = 3 * S * W          # f32 per partition per half (6144)

    def ring(i: int):
        return nc.sync if i % 2 == 0 else nc.scalar

    with tile.TileContext(nc) as tc:
        with (
            tc.tile_pool(name="zin", bufs=PRELOAD + 1) as zin_pool,
            tc.tile_pool(name="wide", bufs=3) as w_pool,
        ):
            zts: list = []

            def issue_load(u: int) -> None:
                zt = zin_pool.tile([CAH, 2 * 3 * CAW], F32)
                zts.append(zt)
                ring(u).dma_start(
                    out=zt[:], in_=zp[u].rearrange("(p x) -> p x", p=CAH)
                )

            for u in range(min(PRELOAD, n_pairs)):
                issue_load(u)

            for u in range(n_pairs):
                zt = zts[u]
                zq = zt[:].rearrange("p (q kc) -> p q kc", q=6)

                w2 = w_pool.tile([CAH, 2 * QH], F32, tag="wide")
                w2v = w2[:].rearrange(
                    "p (q jr kc kr) -> p q jr kc kr", q=6, jr=S, kc=CAW, kr=S
                )
                w2f = w2[:].rearrange("p (q jr k) -> p q jr k", q=6, jr=S)
                ov = op[u].rearrange("(p h x) -> p h x", p=CAH, h=2)
                wh = w2[:].rearrange("p (h x) -> p h x", h=2)

                for h in range(2):
                    q0 = 3 * h
                    # Width-expand x4 (with the 1/4 scale) via a 0-stride
                    # broadcast input; height-replicate jr=1..3 split across
                    # DVE and ACT (gpsimd's tensor_copy is ~4x slower).
                    zb = zq[:, q0 : q0 + 3, :].unsqueeze(3).broadcast_to(
                        [CAH, 3, CAW, S]
                    )
                    nc.vector.tensor_scalar_mul(
                        w2v[:, q0 : q0 + 3, 0, :, :], zb, 0.25
                    )
                    nc.scalar.copy(
                        w2f[:, q0 : q0 + 3, 1, :], w2f[:, q0 : q0 + 3, 0, :]
                    )
                    nc.vector.tensor_copy(
                        w2f[:, q0 : q0 + 3, 2, :], w2f[:, q0 : q0 + 3, 0, :]
                    )
                    nc.scalar.copy(
                        w2f[:, q0 : q0 + 3, 3, :], w2f[:, q0 : q0 + 3, 0, :]
                    )
                    # 3 MiB half-store, 24 KiB contiguous runs both sides.
                    # h parity alternates the two HWDGE rings.
                    ring(h).dma_start(out=ov[:, h, :], in_=wh[:, h, :])

                if u + PRELOAD < n_pairs:
                    issue_load(u + PRELOAD)

    _split_excess_waits(nc)
    return nc


_NC_CACHE: dict[int, bass.Bass] = {}


def _get_nc(b_shard: int = B_SHARD) -> bass.Bass:
    if b_shard not in _NC_CACHE:
        _NC_CACHE[b_shard] = _build_nc(b_shard)
    return _NC_CACHE[b_shard]


def kernel(z: np.ndarray) -> np.ndarray:
    z = np.ascontiguousarray(z, dtype=np.float32)
    assert z.shape == (BATCH, C * CAH * CAW), z.shape
    nc = _get_nc()
    in_maps = [
        {"z": z[i * B_SHARD : (i + 1) * B_SHARD]} for i in range(N_CORES)
    ]
    res = run_bass_kernel_spmd(nc, in_maps, list(range(N_CORES)))
    return np.concatenate(
        [res.results[i]["out"].reshape(B_SHARD, C, H, W) for i in range(N_CORES)],
        axis=0,
    )


# revision 16
# speedup vs baseline: 1.1078x; 1.1078x over previous
"""Haar DWT decoder (2-level inverse, zero details) as a Trainium2 Bass kernel.

out[b, c, j, k] = z[b].reshape(C, 128, 128)[c, j//4, k//4] * 0.25
i.e. a 4x4 nearest-neighbor upsample scaled by 1/4.

Data-parallel over batch: 128 samples -> 16 per core on 8 NeuronCores.

Per-core shape of the problem: read 3 MiB of z, write 48 MiB of output
through 16 SDMA engines at ~26.5 GB/s each (~424 GB/s aggregate), so the
floor is ~122 us of streaming plus the pipeline lead-in.

All DMA (loads and stores) goes through the two HWDGE rings (sync +
scalar). SWDGE (gpsimd) is deliberately unused: its descriptor rings
live on SBUF partitions whose AXI ports are shared with SDMA engines
7/15, and descriptor fetches from those rings are the known cause of
the "engines 7/15 run ~18% slow" straggler mode that adds ~20 us of
tail. HWDGE has no SBUF descriptor ring (and ~0.6 us vs ~1 us
first-byte latency). Loads ride the store rings' FIFO: six are issued
up front, then one after each store — a load is only ~0.5 us of ring
time, and compute stays ~20 us ahead of the ring cadence, so they never
stall the stores.
"""

import numpy as np

import concourse.bass as bass
import concourse.mybir as mybir
import concourse.tile as tile
from concourse.bass_utils import run_bass_kernel_spmd

# The walrus build in this container rejects instructions carrying more than
# one sync-wait command (codegen: "Too many sync wait commands" — observed on
# a Drain with 3 waits and a DMACopy with 2). Tile freely attaches several
# waits to one instruction, so after tracing we split the excess onto NOPs
# inserted just before the instruction on the same engine; sequential
# dispatch on one engine makes that equivalent.
_MAX_WAITS = 1


def _split_excess_waits(nc: bass.Bass) -> None:
    for f in nc.m.functions:
        for bb in f.blocks:
            insns = bb.instructions
            # Iterate over a snapshot; mutate the live list via insert.
            for ins in list(insns):
                si = ins.sync_info
                if si is None or not si.on_wait or len(si.on_wait) <= _MAX_WAITS:
                    continue
                waits = list(si.on_wait)
                keep = waits[-_MAX_WAITS:]
                spill = waits[:-_MAX_WAITS]
                pos = insns.index(ins)
                nops = []
                for i in range(0, len(spill), _MAX_WAITS):
                    nop = nc.engines[ins.engine].nop(nofuse=True).ins
                    # nop() appended itself to the current bb; pull it out.
                    cur = nc.cur_bb.bb.instructions
                    assert cur[-1] is nop
                    cur.pop()
                    nop.sync_info = mybir.SyncInfo(
                        on_wait=spill[i : i + _MAX_WAITS], on_update=[]
                    )
                    nops.append(nop)
                insns[pos:pos] = nops
                ins.sync_info = mybir.SyncInfo(
                    on_wait=keep, on_update=list(si.on_update)
                )

# Problem constants (hardcoded: module config out_shape=(3,512,512), levels=2)
BATCH = 128
C = 3
CAH = 128  # coarse-approximation spatial dims
CAW = 128
S = 4      # 2**levels upsample factor
H = 512
W = 512
N_CORES = 8
B_SHARD = BATCH // N_CORES  # 16

# Loads issued ahead of the compute loop (ring FIFO keeps them ~6 samples
# ahead of the stores, which is far more than compute needs).
PRELOAD = 6

F32 = mybir.dt.float32


def _build_nc(b_shard: int = B_SHARD) -> bass.Bass:
    nc = bass.Bass("TRN2", target_bir_lowering=False, debug=False)
    z = nc.dram_tensor("z", [b_shard, C * CAH * CAW], F32, kind="ExternalInput").ap()
    # Output is declared FLAT per sample and reshaped to (C, H, W) in numpy:
    # a coarse row r = c*128+jc owns exactly the 2048 contiguous output
    # floats at offset 2048*r, so partition p holding rows 3p..3p+2 stores a
    # fully-contiguous 24 KiB run — 3x bigger descriptors than the
    # channel-major layout, and the load becomes perfectly contiguous too
    # (1536 B runs instead of the transpose layout's 512 B).
    out = nc.dram_tensor("out", [b_shard, C * H * W], F32, kind="ExternalOutput").ap()

    def ring(i: int):
        return nc.sync if i % 2 == 0 else nc.scalar

    with tile.TileContext(nc) as tc:
        with (
            tc.tile_pool(name="zin", bufs=PRELOAD + 1) as zin_pool,
            tc.tile_pool(name="wide", bufs=6) as w_pool,
        ):
            zts: list = []

            def issue_load(b: int) -> None:
                # Fully-contiguous load: partition p gets z[b][384p:384p+384]
                # (= coarse rows 3p..3p+2 in (c*128+jc) order).
                zt = zin_pool.tile([CAH, 3 * CAW], F32)
                zts.append(zt)
                ring(b).dma_start(
                    out=zt[:], in_=z[b].rearrange("(p x) -> p x", p=CAH)
                )

            for b in range(min(PRELOAD, b_shard)):
                issue_load(b)

            for b in range(b_shard):
                zt = zts[b]
                zq = zt[:].rearrange("p (q kc) -> p q kc", q=3)

                # Partition p materializes its 3 coarse rows' upsampled
                # output: free layout (q, jr, kc, kr), 24 KiB per partition,
                # which IS the flat output byte range [24KiB*p, 24KiB*(p+1)).
                w2 = w_pool.tile([CAH, 3 * S * W], F32, tag="wide")
                w2v = w2[:].rearrange(
                    "p (q jr kc kr) -> p q jr kc kr", q=3, jr=S, kc=CAW, kr=S
                )
                w2f = w2[:].rearrange("p (q jr k) -> p q jr k", q=3, jr=S)

                # Width-expand x4 (with the 1/4 scale) via a 0-stride
                # broadcast input; height-replicate jr=1..3 split across DVE
                # and ACT (gpsimd's tensor_copy is ~4x slower — don't).
                zb = zq.unsqueeze(3).broadcast_to([CAH, 3, CAW, S])
                nc.vector.tensor_scalar_mul(w2v[:, :, 0, :, :], zb, 0.25)
                nc.scalar.copy(w2f[:, :, 1, :], w2f[:, :, 0, :])
                nc.vector.tensor_copy(w2f[:, :, 2, :], w2f[:, :, 0, :])
                nc.scalar.copy(w2f[:, :, 3, :], w2f[:, :, 0, :])

                # One fully-contiguous 3 MiB store per sample, 24 KiB
                # descriptor runs on both sides; alternate HWDGE rings.
                ring(b).dma_start(
                    out=out[b].rearrange("(p x) -> p x", p=CAH), in_=w2[:]
                )

                # The next load goes on the same ring right after this store:
                # it drains ~6 samples before compute needs it.
                if b + PRELOAD < b_shard:
                    issue_load(b + PRELOAD)

    _split_excess_waits(nc)
    return nc


_NC_CACHE: dict[int, bass.Bass] = {}


def _get_nc(b_shard: int = B_SHARD) -> bass.Bass:
    if b_shard not in _NC_CACHE:
        _NC_CACHE[b_shard] = _build_nc(b_shard)
    return _NC_CACHE[b_shard]


def kernel(z: np.ndarray) -> np.ndarray:
    z = np.ascontiguousarray(z, dtype=np.float32)
    assert z.shape == (BATCH, C * CAH * CAW), z.shape
    nc = _get_nc()
    in_maps = [
        {"z": z[i * B_SHARD : (i + 1) * B_SHARD]} for i in range(N_CORES)
    ]
    res = run_bass_kernel_spmd(nc, in_maps, list(range(N_CORES)))
    return np.concatenate(
        [res.results[i]["out"].reshape(B_SHARD, C, H, W) for i in range(N_CORES)],
        axis=0,
    )
